# revision 1
# baseline (speedup 1.0000x reference)
"""Trainium2 Bass kernel for nn_ConstrainLoss (weighted logsumexp over a
Gaussian-kernel cost matrix, dotted with row weights -> scalar).

Math:
    sq_ij = |x_i - xo_j|^2          (relu clamp in the reference never fires:
                                     min pairwise sq on this data is ~5.2)
    C_ij  = -2*sq_ij + log(w_obs_j)          (inv_two_s2 == 2.0)
          = 4*x_i.xo_j + a_j + b_i
      a_j = -2*|xo_j|^2 + log(w_obs_j)
      b_i = -2*|x_i|^2            (pulls out of the LSE entirely -> host term)
    out   = -sum_i x_w_i * (b_i + logsumexp_j(T_ij)),  T_ij = 4*x_i.xo_j + a_j

Device kernel (per core, rows sharded 2048/core):
    T tile: one K=98 bf16 matmul per 512-column chunk. The 98 contraction
      rows implement a compensated (hi/lo split) product plus the a_j bias:
        rows  0-31: hi(4x)  . hi(xo)
        rows 32-63: hi(4x)  . lo(xo)
        rows 64-95: lo(4x)  . hi(xo)
        row  96/97: 1 . a_hi, 1 . a_lo
      Max |T| error ~1.3e-3 (vs 0.03 for fp32r, ~1 for plain bf16) while the
      PE streams 1 column/cycle (4x faster than fp32's 4 cycles/row).
    shift_i: max over the first 512 columns of the row (DVE reduce, negated).
      A valid LSE shift: max_j T - shift <= ~69 on this data (verified), so
      exp stays in fp32 range with >4 orders of margin.
    s_g: sum_j exp(T_ij - shift_i) per 2048-column group -- a single ScalarE
      activation with fused accum_out, reading 4 psum banks in place.
    lse_i = shift_i + ln(sum_g s_g) computed as Ln(S*2^-63) + 63*ln2: the ACT
      Ln LUT is only accurate on ~[5e-20, 2e19] and S can reach ~1e34.
    acc_p = sum over this partition's rows of (shift+lnS')*x_w    -> [128,1]

Host: result = -(sum_cores sum(acc) + sum_i b_i*x_w_i + 63*ln2*sum_i x_w_i)
"""

import sys

if "/opt/trn_rl_repo" not in sys.path:
    sys.path.insert(0, "/opt/trn_rl_repo")

import re
from contextlib import ExitStack

import ml_dtypes
import numpy as np

import bass_rust
import concourse.bass as bass
import concourse.tile as tile
from concourse import mybir
from concourse.bass_utils import run_bass_kernel_spmd
from concourse.tile import ScopedClock, TileContext


def _patched_drain_and_barrier(self, tick_clock, wait_clock):
    """The walrus build in this container rejects >1 sync wait on one
    instruction ("Too many sync wait commands" on Tile's kernel-tail drain).
    Split the tail-drain waits onto individual SP nops, one wait each."""
    gc = tick_clock.global_clock
    ticks = [int(s) for s in re.findall(r"\d+", repr(gc))]
    for i, t in enumerate(ticks):
        if t > 0:
            nop = self.nc.sync.nop(hint="split_wait", nofuse=True)
            vc = bass_rust.VectorClock()
            vc.require_at_least(i, t)
            wait_clock.add_sem_waits(nop.ins, ScopedClock({None: vc}))
    self.nc.sync.drain()
    self.nc.all_engine_barrier()
    assert self.sems is not None
    popped = self.nc._tile_sem_poison_stack.pop()
    assert popped is self._sem_poison
    self.nc.clear_and_free_semaphores(list(self.sems.allocated().values()))
    self.nc.all_engine_barrier()


TileContext._drain_and_barrier = _patched_drain_and_barrier

_MAX_WAITS = 1  # this walrus build rejects >1 sync wait per instruction


def _split_excess_waits(nc):
    """Move excess sync waits (beyond _MAX_WAITS) from any instruction onto
    freshly inserted same-engine nops placed immediately before it. The
    engine executes the nops (waiting) first, so semantics are unchanged."""
    counter = [0]
    for f in nc.m.functions:
        for blk in f.blocks:
            il = blk.instructions  # live list
            i = 0
            while i < len(il):
                ins = il[i]
                si = ins.sync_info
                if si is not None and len(si.on_wait) > _MAX_WAITS:
                    waits = list(si.on_wait)
                    keep = waits[-_MAX_WAITS:]
                    excess = waits[: -_MAX_WAITS]
                    pos = i
                    for j in range(0, len(excess), _MAX_WAITS):
                        counter[0] += 1
                        nop = mybir.InstNoOp(
                            name=f"I-splitw{counter[0]}", ins=[], outs=[]
                        )
                        nop.engine = ins.engine
                        nop.sync_info = mybir.SyncInfo(
                            on_wait=excess[j : j + _MAX_WAITS], on_update=[]
                        )
                        il.insert(pos, nop)
                        pos += 1
                        i += 1
                    ins.sync_info = mybir.SyncInfo(
                        on_wait=keep, on_update=list(si.on_update)
                    )
                i += 1


N, M, D = 16384, 16384, 32
NCORES = 8
N_LOC = N // NCORES  # 2048 rows per core
KK = 3 * D + 2  # 98: hi*hi, hi*lo, lo*hi splits + a_hi + a_lo rows
BLK = 128  # rows per block (psum partitions)
NBLK = N_LOC // BLK  # 16
CHUNK = 512  # matmul free dim (one psum bank fp32)
GROUP = 2048  # columns per ScalarE exp+accum instruction (4 banks)
NGROUP = M // GROUP  # 8
SEED_W = 512  # seed max over first SEED_W columns

F32 = mybir.dt.float32
BF16 = mybir.dt.bfloat16

_cache = {}


def _build_bass():
    nc = bass.Bass()
    xT_d = nc.declare_dram_parameter("xT", [KK, N_LOC], BF16, isOutput=False)
    xoT_d = nc.declare_dram_parameter("xoT", [KK, M], BF16, isOutput=False)
    negsh_d = nc.declare_dram_parameter("negsh", [BLK, NBLK], F32, isOutput=False)
    s_d = nc.declare_dram_parameter("s_out", [BLK, NBLK * NGROUP], F32, isOutput=True)

    with tile.TileContext(nc) as tc, ExitStack() as ctx:
        singles = ctx.enter_context(tc.tile_pool(name="singles", bufs=1))
        small = ctx.enter_context(tc.tile_pool(name="small", bufs=4))
        psp = ctx.enter_context(tc.tile_pool(name="ps", bufs=2, space="PSUM"))

        xo_sb = singles.tile([128, M], BF16)
        x_sb = singles.tile([128, N_LOC], BF16)
        s_full = singles.tile([BLK, NBLK * NGROUP], F32)
        negsh_full = singles.tile([BLK, NBLK], F32)

        # Spread input DMAs across engine queues so they land in parallel;
        # the first matmuls depend only on x + negsh + xo piece 0.
        nc.sync.dma_start(out=negsh_full, in_=negsh_d[:, :])
        nc.sync.dma_start(out=x_sb[0:KK, :], in_=xT_d[:, :])
        NPIECE = 8
        PW = M // NPIECE
        dma_engines = [nc.sync, nc.gpsimd]
        for p in range(NPIECE):
            dma_engines[p % len(dma_engines)].dma_start(
                out=xo_sb[0:KK, p * PW : (p + 1) * PW],
                in_=xoT_d[:, p * PW : (p + 1) * PW],
            )

        for b in range(NBLK):
            negsh = negsh_full[:, b : b + 1]
            s_all = s_full[:, b * NGROUP : (b + 1) * NGROUP]
            for g in range(NGROUP):
                ps = psp.tile([BLK, GROUP], F32, tag="ps")
                for c in range(GROUP // CHUNK):
                    j0 = g * GROUP + c * CHUNK
                    nc.tensor.matmul(
                        out=ps[:, c * CHUNK : (c + 1) * CHUNK],
                        lhsT=x_sb[0:KK, b * BLK : (b + 1) * BLK],
                        rhs=xo_sb[0:KK, j0 : j0 + CHUNK],
                        start=True,
                        stop=True,
                    )
                nc.scalar.activation(
                    out=ps,
                    in_=ps,
                    func=mybir.ActivationFunctionType.Exp,
                    bias=negsh,
                    scale=1.0,
                    accum_out=s_all[:, g : g + 1],
                )
            nc.sync.dma_start(
                out=s_d[:, b * NGROUP : (b + 1) * NGROUP],
                in_=s_all,
            )

    _split_excess_waits(nc)
    return nc


def _get_nc():
    if "nc" not in _cache:
        _cache["nc"] = _build_bass()
    return _cache["nc"]


def _bf_split(v):
    hi = v.astype(ml_dtypes.bfloat16)
    lo = (v - hi.astype(np.float32)).astype(ml_dtypes.bfloat16)
    return hi, lo


def _prep_inputs(x, x_w, x_obs, x_obs_w):
    x = np.ascontiguousarray(x, dtype=np.float32)
    x_w = np.ascontiguousarray(x_w, dtype=np.float32)
    x_obs = np.ascontiguousarray(x_obs, dtype=np.float32)
    x_obs_w = np.ascontiguousarray(x_obs_w, dtype=np.float32)

    c = np.sum(x_obs * x_obs, axis=1, dtype=np.float32)
    a = (-2.0 * c + np.log(x_obs_w)).astype(np.float32)
    a_hi, a_lo = _bf_split(a)
    xo_hi, xo_lo = _bf_split(x_obs)
    xoT = np.empty((KK, M), dtype=ml_dtypes.bfloat16)
    xoT[0:D] = xo_hi.T
    xoT[D : 2 * D] = xo_lo.T
    xoT[2 * D : 3 * D] = xo_hi.T
    xoT[3 * D] = a_hi
    xoT[3 * D + 1] = a_lo

    x4 = 4.0 * x
    x_hi, x_lo = _bf_split(x4)

    # Host-side LSE shift: exact max of T over the first SEED_W columns.
    # Any shift within ~80 of the row max is numerically valid; on this data
    # max_j T - shift <= ~69 (verified), leaving >4 orders of fp32 margin.
    T_seed = (
        4.0 * (x @ x_obs[:SEED_W].T) + a[None, :SEED_W]
    ).astype(np.float32)
    shift = T_seed.max(axis=1)  # [N]

    one = np.ones((1,), dtype=ml_dtypes.bfloat16)
    in_maps = []
    for core in range(NCORES):
        sl = slice(core * N_LOC, (core + 1) * N_LOC)
        xT = np.empty((KK, N_LOC), dtype=ml_dtypes.bfloat16)
        xT[0:D] = x_hi[sl].T
        xT[D : 2 * D] = x_hi[sl].T
        xT[2 * D : 3 * D] = x_lo[sl].T
        xT[3 * D] = one
        xT[3 * D + 1] = one
        negsh = np.ascontiguousarray(
            -shift[sl].reshape(NBLK, BLK).T, dtype=np.float32
        )
        in_maps.append({"xT": xT, "xoT": xoT, "negsh": negsh})
    return in_maps, shift


def kernel(x, x_w, x_obs, x_obs_w, _trace=False, _tmpdir=None):
    nc = _get_nc()
    in_maps, shift = _prep_inputs(x, x_w, x_obs, x_obs_w)
    res = run_bass_kernel_spmd(
        nc,
        in_maps,
        core_ids=list(range(NCORES)),
        trace=_trace,
        tmpdir=_tmpdir,
    )
    _cache["last_results"] = res
    # host epilogue (fp64): lse_i = shift_i + log(sum_g s_ig) + b_i
    x = np.ascontiguousarray(x, dtype=np.float32)
    x_w64 = np.ascontiguousarray(x_w, dtype=np.float32).astype(np.float64)
    r = np.sum(x.astype(np.float64) * x, axis=1)
    total = float(np.dot(-2.0 * r, x_w64))
    for core in range(NCORES):
        out = res.results[core]
        S = (
            out["s_out"]
            .astype(np.float64)
            .reshape(BLK, NBLK, NGROUP)
            .sum(axis=2)
        )  # [128 rows, 16 blocks]
        sl = slice(core * N_LOC, (core + 1) * N_LOC)
        sh = shift[sl].astype(np.float64).reshape(NBLK, BLK).T
        lse = sh + np.log(S)
        w_arr = x_w64[sl].reshape(NBLK, BLK).T
        total += float((lse * w_arr).sum())
    return np.asarray(-total, dtype=np.float32)



# revision 2
# speedup vs baseline: 1.1608x; 1.1608x over previous
"""Trainium2 Bass kernel for nn_ConstrainLoss (weighted logsumexp over a
Gaussian-kernel cost matrix, dotted with row weights -> scalar).

Math:
    sq_ij = |x_i - xo_j|^2
    C_ij  = -2*sq_ij + log(w_obs_j)     (inv_two_s2 == 2.0)
          = 4*x_i.xo_j + a_j + b_i
      a_j = -2*|xo_j|^2 + log(w_obs_j)
      b_i = -2*|x_i|^2                  (pulled out of the LSE -> host term)
    out   = -sum_i x_w_i * (b_i + shift_i + log S_i)
      S_i = sum_j exp(T_ij - shift_i),  T_ij = 4*x_i.xo_j + a_j
    shift_i = max of T over the first 512 columns (host seed max; verified
      max_j T - shift <= ~69 on this data, so f32 never overflows).

Device kernel (per core, 2048 rows = 16 blocks of 128 partitions):
  The exp+sum of 2048x16384 elements is the bottleneck, so it is split
  across TWO engines working in parallel on disjoint column ranges:

  * ACT path (columns 0..10751, 7 chunks of 1536 per block):
      one K=102 bf16 matmul trio -> psum f32 T tile; ScalarE activation
      Exp with bias=-shift and fused accum_out -> per-chunk partial sums.
  * DVE path (columns 10752..16383, 11 chunks of 512 per block):
      the matmul directly emits v_ij = A*(T_ij - shift_i) + B (Schraudolph
      integer scale, A = 2^23*log2(e), B = 127*2^23 - C_BIAS) using a
      second pre-scaled lhsT. VectorE tensor_scalar max(v,0) with int32
      output converts (round-to-nearest) to the Schraudolph integer whose
      bit pattern IS ~exp(T-shift) in f32; clamped lanes give bits 0 ->
      +0.0 (t-shift < -88, contributes nothing).  A reduce_sum over the
      bitcast-f32 staging tile yields the partial sums. Max per-element
      error ~3% (mean tuned out via C_BIAS); contributes <2e-4 to the
      final loss on this data (verified on host emulation).

  The 102 contraction rows: 96 compensated (hi/lo bf16) product rows
  shared layout + a_j / A*a_j bias rows + per-row q_i = -A*shift_i + B
  rows, selected per path by zeroing unused lhsT rows.

PSUM: ACT 2x[128,1536] (6 banks) + DVE 2x[128,512] (2 banks) = 8 banks.
Host: epilogue lse/log/dot in f64 (O(N) work), as in the baseline.
"""

import sys

if "/opt/trn_rl_repo" not in sys.path:
    sys.path.insert(0, "/opt/trn_rl_repo")

import re
from contextlib import ExitStack

import ml_dtypes
import numpy as np

import bass_rust
import concourse.bass as bass
import concourse.tile as tile
from concourse import mybir
from concourse.bass_utils import run_bass_kernel_spmd
from concourse.tile import ScopedClock, TileContext


def _patched_drain_and_barrier(self, tick_clock, wait_clock):
    """The walrus build in this container rejects >1 sync wait on one
    instruction ("Too many sync wait commands" on Tile's kernel-tail drain).
    Split the tail-drain waits onto individual SP nops, one wait each."""
    gc = tick_clock.global_clock
    ticks = [int(s) for s in re.findall(r"\d+", repr(gc))]
    for i, t in enumerate(ticks):
        if t > 0:
            nop = self.nc.sync.nop(hint="split_wait", nofuse=True)
            vc = bass_rust.VectorClock()
            vc.require_at_least(i, t)
            wait_clock.add_sem_waits(nop.ins, ScopedClock({None: vc}))
    self.nc.sync.drain()
    self.nc.all_engine_barrier()
    assert self.sems is not None
    popped = self.nc._tile_sem_poison_stack.pop()
    assert popped is self._sem_poison
    self.nc.clear_and_free_semaphores(list(self.sems.allocated().values()))
    self.nc.all_engine_barrier()


TileContext._drain_and_barrier = _patched_drain_and_barrier

_MAX_WAITS = 1  # this walrus build rejects >1 sync wait per instruction


def _split_excess_waits(nc):
    """Move excess sync waits (beyond _MAX_WAITS) from any instruction onto
    freshly inserted same-engine nops placed immediately before it. The
    engine executes the nops (waiting) first, so semantics are unchanged."""
    counter = [0]
    for f in nc.m.functions:
        for blk in f.blocks:
            il = blk.instructions  # live list
            i = 0
            while i < len(il):
                ins = il[i]
                si = ins.sync_info
                if si is not None and len(si.on_wait) > _MAX_WAITS:
                    waits = list(si.on_wait)
                    keep = waits[-_MAX_WAITS:]
                    excess = waits[: -_MAX_WAITS]
                    pos = i
                    for j in range(0, len(excess), _MAX_WAITS):
                        counter[0] += 1
                        nop = mybir.InstNoOp(
                            name=f"I-splitw{counter[0]}", ins=[], outs=[]
                        )
                        nop.engine = ins.engine
                        nop.sync_info = mybir.SyncInfo(
                            on_wait=excess[j : j + _MAX_WAITS], on_update=[]
                        )
                        il.insert(pos, nop)
                        pos += 1
                        i += 1
                    ins.sync_info = mybir.SyncInfo(
                        on_wait=keep, on_update=list(si.on_update)
                    )
                i += 1


N, M, D = 16384, 16384, 32
NCORES = 8
N_LOC = N // NCORES  # 2048 rows per core
KK = 102  # 96 product rows + a/a2 bias rows + 2 per-row q rows
BLK = 128
NBLK = N_LOC // BLK  # 16
SEED_W = 512

ACT_CHUNK = 1536  # 3 psum banks
ACT_PER_BLK = 7  # cols 0..10751
DVE_CHUNK = 512  # 1 psum bank
DVE_PER_BLK = 11  # cols 10752..16383
ACT_COLS = ACT_CHUNK * ACT_PER_BLK  # 10752
DVE_RED = (4, 4, 3)  # ts chunks per reduce -> 3 reduce slots/block

A_SCHRAUD = float(np.float64(2**23) * np.log2(np.e))  # 12102203.16
C_BIAS = 485000.0
B_SCHRAUD = 127.0 * 2**23 - C_BIAS

F32 = mybir.dt.float32
I32 = mybir.dt.int32
BF16 = mybir.dt.bfloat16

_cache = {}


def _build_bass():
    nc = bass.Bass()
    xN_d = nc.declare_dram_parameter("xN", [KK, N_LOC], BF16, isOutput=False)
    xS_d = nc.declare_dram_parameter("xS", [KK, N_LOC], BF16, isOutput=False)
    xoT_d = nc.declare_dram_parameter("xoT", [KK, M], BF16, isOutput=False)
    negsh_d = nc.declare_dram_parameter("negsh", [BLK, NBLK], F32, isOutput=False)
    sA_d = nc.declare_dram_parameter(
        "sA_out", [BLK, NBLK * ACT_PER_BLK], F32, isOutput=True
    )
    sD_d = nc.declare_dram_parameter(
        "sD_out", [BLK, NBLK * len(DVE_RED)], F32, isOutput=True
    )

    with tile.TileContext(nc) as tc, ExitStack() as ctx:
        singles = ctx.enter_context(tc.tile_pool(name="singles", bufs=1))
        psA = ctx.enter_context(tc.tile_pool(name="psA", bufs=2, space="PSUM"))
        psD = ctx.enter_context(tc.tile_pool(name="psD", bufs=2, space="PSUM"))

        xo_sb = singles.tile([128, M], BF16)
        xN_sb = singles.tile([128, N_LOC], BF16)
        xS_sb = singles.tile([128, N_LOC], BF16)
        negsh_full = singles.tile([BLK, NBLK], F32)
        sA = singles.tile([BLK, NBLK * ACT_PER_BLK], F32)
        sD = singles.tile([BLK, NBLK * len(DVE_RED)], F32)
        stage = singles.tile([BLK, DVE_CHUNK * DVE_PER_BLK], I32)  # 5632

        # Input DMAs. ACT path needs negsh+xN+cols[0:1536); DVE path needs
        # xS+cols[10752:11264). Split xo by path region across two queue
        # engines so both paths unblock early.
        nc.sync.dma_start(out=negsh_full, in_=negsh_d[:, :])
        nc.sync.dma_start(out=xN_sb[0:KK, :], in_=xN_d[:, :])
        nc.gpsimd.dma_start(out=xS_sb[0:KK, :], in_=xS_d[:, :])
        a_bounds = [0, 2048, 4096, 6144, 8192, ACT_COLS]
        for lo, hi in zip(a_bounds[:-1], a_bounds[1:]):
            nc.sync.dma_start(
                out=xo_sb[0:KK, lo:hi], in_=xoT_d[:, lo:hi]
            )
        d_bounds = [ACT_COLS, 12800, 14848, M]
        for lo, hi in zip(d_bounds[:-1], d_bounds[1:]):
            nc.gpsimd.dma_start(
                out=xo_sb[0:KK, lo:hi], in_=xoT_d[:, lo:hi]
            )

        for b in range(NBLK):
            lhsN = xN_sb[0:KK, b * BLK : (b + 1) * BLK]
            lhsS = xS_sb[0:KK, b * BLK : (b + 1) * BLK]
            negsh = negsh_full[:, b : b + 1]

            # interleave ACT/DVE chunk issue so both consumers start early
            order = []
            for i in range(ACT_PER_BLK):
                order.append(("A", i))
                order.append(("D", i))
            for i in range(ACT_PER_BLK, DVE_PER_BLK):
                order.append(("D", i))

            ts_done = 0
            red_idx = 0
            for kind, i in order:
                if kind == "A":
                    ps = psA.tile([BLK, ACT_CHUNK], F32, tag="psA")
                    for c in range(ACT_CHUNK // 512):
                        j0 = i * ACT_CHUNK + c * 512
                        nc.tensor.matmul(
                            out=ps[:, c * 512 : (c + 1) * 512],
                            lhsT=lhsN,
                            rhs=xo_sb[0:KK, j0 : j0 + 512],
                            start=True,
                            stop=True,
                        )
                    nc.scalar.activation(
                        out=ps,
                        in_=ps,
                        func=mybir.ActivationFunctionType.Exp,
                        bias=negsh,
                        scale=1.0,
                        accum_out=sA[:, b * ACT_PER_BLK + i : b * ACT_PER_BLK + i + 1],
                    )
                else:
                    ps = psD.tile([BLK, DVE_CHUNK], F32, tag="psD")
                    j0 = ACT_COLS + i * DVE_CHUNK
                    nc.tensor.matmul(
                        out=ps,
                        lhsT=lhsS,
                        rhs=xo_sb[0:KK, j0 : j0 + 512],
                        start=True,
                        stop=True,
                    )
                    nc.vector.tensor_scalar(
                        out=stage[:, i * DVE_CHUNK : (i + 1) * DVE_CHUNK],
                        in0=ps,
                        scalar1=0.0,
                        scalar2=None,
                        op0=mybir.AluOpType.max,
                    )
                    ts_done += 1
                    # emit a reduce as soon as its chunk group completes
                    want = sum(DVE_RED[: red_idx + 1])
                    if ts_done == want:
                        r0 = sum(DVE_RED[:red_idx]) * DVE_CHUNK
                        r1 = want * DVE_CHUNK
                        slot = b * len(DVE_RED) + red_idx
                        nc.vector.tensor_reduce(
                            out=sD[:, slot : slot + 1],
                            in_=stage.bitcast(F32)[:, r0:r1],
                            axis=mybir.AxisListType.X,
                            op=mybir.AluOpType.add,
                        )
                        red_idx += 1

        nc.sync.dma_start(out=sA_d[:, :], in_=sA)
        nc.sync.dma_start(out=sD_d[:, :], in_=sD)

    _split_excess_waits(nc)
    return nc


def _get_nc():
    if "nc" not in _cache:
        _cache["nc"] = _build_bass()
    return _cache["nc"]


def _bf_split(v):
    hi = v.astype(ml_dtypes.bfloat16)
    lo = (v - hi.astype(np.float32)).astype(ml_dtypes.bfloat16)
    return hi, lo


def _prep_inputs(x, x_w, x_obs, x_obs_w):
    x = np.ascontiguousarray(x, dtype=np.float32)
    x_obs = np.ascontiguousarray(x_obs, dtype=np.float32)
    x_obs_w = np.ascontiguousarray(x_obs_w, dtype=np.float32)

    c = np.sum(x_obs * x_obs, axis=1, dtype=np.float32)
    a = (-2.0 * c + np.log(x_obs_w)).astype(np.float32)
    a_hi, a_lo = _bf_split(a)
    a2 = (np.float64(A_SCHRAUD) * a.astype(np.float64)).astype(np.float32)
    a2_hi, a2_lo = _bf_split(a2)
    xo_hi, xo_lo = _bf_split(x_obs)

    one = np.ones(M, dtype=ml_dtypes.bfloat16)
    xoT = np.empty((KK, M), dtype=ml_dtypes.bfloat16)
    xoT[0:D] = xo_hi.T
    xoT[D : 2 * D] = xo_lo.T
    xoT[2 * D : 3 * D] = xo_hi.T
    xoT[96] = a_hi
    xoT[97] = a_lo
    xoT[98] = a2_hi
    xoT[99] = a2_lo
    xoT[100] = one
    xoT[101] = one

    x4 = 4.0 * x
    x_hi, x_lo = _bf_split(x4)
    sx = (4.0 * np.float64(A_SCHRAUD) * x.astype(np.float64)).astype(np.float32)
    sx_hi, sx_lo = _bf_split(sx)

    # Host-side LSE shift: exact max of T over the first SEED_W columns.
    # max_j T - shift <= ~69 on this data (verified), so the ACT exp stays
    # in f32 range and the Schraudolph integer stays < 2^31.
    T_seed = (4.0 * (x @ x_obs[:SEED_W].T) + a[None, :SEED_W]).astype(np.float32)
    shift = T_seed.max(axis=1)  # [N]
    q = (
        -np.float64(A_SCHRAUD) * shift.astype(np.float64) + B_SCHRAUD
    ).astype(np.float32)
    q_hi, q_lo = _bf_split(q)

    oneb = np.ones(N_LOC, dtype=ml_dtypes.bfloat16)
    zerob = np.zeros(N_LOC, dtype=ml_dtypes.bfloat16)
    in_maps = []
    for core in range(NCORES):
        sl = slice(core * N_LOC, (core + 1) * N_LOC)
        xN = np.empty((KK, N_LOC), dtype=ml_dtypes.bfloat16)
        xN[0:D] = x_hi[sl].T
        xN[D : 2 * D] = x_hi[sl].T
        xN[2 * D : 3 * D] = x_lo[sl].T
        xN[96] = oneb
        xN[97] = oneb
        xN[98:102] = zerob
        xS = np.empty((KK, N_LOC), dtype=ml_dtypes.bfloat16)
        xS[0:D] = sx_hi[sl].T
        xS[D : 2 * D] = sx_hi[sl].T
        xS[2 * D : 3 * D] = sx_lo[sl].T
        xS[96:98] = zerob
        xS[98] = oneb
        xS[99] = oneb
        xS[100] = q_hi[sl]
        xS[101] = q_lo[sl]
        negsh = np.ascontiguousarray(
            -shift[sl].reshape(NBLK, BLK).T, dtype=np.float32
        )
        in_maps.append({"xN": xN, "xS": xS, "xoT": xoT, "negsh": negsh})
    return in_maps, shift


def kernel(x, x_w, x_obs, x_obs_w, _trace=False, _tmpdir=None):
    nc = _get_nc()
    in_maps, shift = _prep_inputs(x, x_w, x_obs, x_obs_w)
    res = run_bass_kernel_spmd(
        nc,
        in_maps,
        core_ids=list(range(NCORES)),
        trace=_trace,
        tmpdir=_tmpdir,
    )
    _cache["last_results"] = res
    # host epilogue (f64): lse_i = shift_i + log(S_i) + b_i
    x = np.ascontiguousarray(x, dtype=np.float32)
    x_w64 = np.ascontiguousarray(x_w, dtype=np.float32).astype(np.float64)
    r = np.sum(x.astype(np.float64) * x, axis=1)
    total = float(np.dot(-2.0 * r, x_w64))
    for core in range(NCORES):
        out = res.results[core]
        S = (
            out["sA_out"]
            .astype(np.float64)
            .reshape(BLK, NBLK, ACT_PER_BLK)
            .sum(axis=2)
        )
        S += (
            out["sD_out"]
            .astype(np.float64)
            .reshape(BLK, NBLK, len(DVE_RED))
            .sum(axis=2)
        )
        sl = slice(core * N_LOC, (core + 1) * N_LOC)
        sh = shift[sl].astype(np.float64).reshape(NBLK, BLK).T
        lse = sh + np.log(S)
        w_arr = x_w64[sl].reshape(NBLK, BLK).T
        total += float((lse * w_arr).sum())
    return np.asarray(-total, dtype=np.float32)
